# revision 1
# baseline (speedup 1.0000x reference)
"""Trainium2 Bass kernel for nn_DGRACL_58523224375313 (retrieval_knn).

Pipeline (8 NeuronCores, pool dim M sharded 12500/core):
  Stage 1 (per core): normalize pool slab + queries on device, score all
    1024 queries against the local slab (PE matmul), apply time decay
    (DVE |dt| + ACT exp + DVE mul), pack (quantized score | local index)
    into fp32-comparable keys, per-512-tile top-8 (DVE max8), merge to
    local top-16 per query chunk.
  AllToAll: each core sends its top-16 keys for query-chunk d to core d.
  Stage 2 (per core, its 128 queries): repack keys with global indices,
    top-40 of the 128-candidate union, gather candidate rows (indirect
    DMA), exact fp32 rescore (cos-sim * exp(-lam*|dt|)), final top-7,
    GCN-fusion via linearity (fused = (sum_k u_k * demo_k)/7 @ W + b),
    anomaly score out = 0.6*(1-cos) + 0.4*l2.
Host only shards/pads/concats and feeds constants.
"""
import sys

sys.path.insert(0, "/opt/trn_rl_repo")

from contextlib import ExitStack

import numpy as np

import concourse.bass as bass
import concourse.bacc as bacc
import concourse.mybir as mybir
import concourse.tile as tile
from concourse.masks import make_identity

dt = mybir.dt
A = mybir.AluOpType
AF = mybir.ActivationFunctionType

# ---- geometry -------------------------------------------------------------
N, D, K = 1024, 256, 7
M = 100000
NCORES = 8
MLOC = M // NCORES           # 12500
MPAD = 12544                 # = 98 * 128
QCH = N // 128               # 8 query chunks of 128 == NCORES
AUGC = 258                   # pool_aug row: 256 emb + time + pad
ALPHA, BETA = 0.6, 0.4
EPS = 1e-8
NEG = -3.0e38

MASK_LOCAL = 0xFFFFC000      # keep 9 mantissa bits, low 14 bits for index
MASK_GLOBAL = 0xFFFE0000     # keep 6 mantissa bits, low 17 bits for index
NCAND = 40                   # rescored candidates per query

# GCN linearity: fused = (sum_k u_k * demos_k)/7 @ W + b
_deg = np.ones(K, np.float32); _deg[1:] = 2.0
_dinv = (1.0 / np.sqrt(_deg)).astype(np.float32)
_selfc = _dinv * _dinv
_edgec = _dinv[:-1] * _dinv[1:]
U_COEF = (_selfc + np.concatenate([_edgec, [0.0]]).astype(np.float32)) / K


def _mtiles(mpad):
    ts, off = [], 0
    while off < mpad:
        s = min(512, mpad - off)
        ts.append((off, s))
        off += s
    return ts


def build_program(mloc=MLOC, mpad=MPAD, ncand=NCAND, fp32r=False, mtotal=M,
                  use_cc=True, stage2=True, s2depth=99):
    nc = bacc.Bacc("TRN2", target_bir_lowering=False, debug=False,
                   num_devices=NCORES)
    f32, u32 = dt.float32, dt.uint32
    mm_dt = dt.float32r if fp32r else dt.float32

    # ---- I/O ---------------------------------------------------------------
    slab = nc.dram_tensor("slab", [mpad, D], f32, kind="ExternalInput")
    slab_pt = nc.dram_tensor("slab_pt", [mpad], f32, kind="ExternalInput")
    pool_aug = nc.dram_tensor("pool_aug", [mtotal, AUGC], f32, kind="ExternalInput")
    queries = nc.dram_tensor("queries", [N, D], f32, kind="ExternalInput")
    qtimes = nc.dram_tensor("qtimes", [N], f32, kind="ExternalInput")
    qslice = nc.dram_tensor("qslice", [128, D], f32, kind="ExternalInput")
    qtslice = nc.dram_tensor("qtslice", [128], f32, kind="ExternalInput")
    lam_in = nc.dram_tensor("lam", [1], f32, kind="ExternalInput")
    W_in = nc.dram_tensor("W", [D, D], f32, kind="ExternalInput")
    b_in = nc.dram_tensor("b", [D], f32, kind="ExternalInput")
    idxcode = nc.dram_tensor("idxcode", [mpad], u32, kind="ExternalInput")
    iotaC = nc.dram_tensor("iotaC", [ncand], f32, kind="ExternalInput")
    out_d = nc.dram_tensor("out", [128], f32, kind="ExternalOutput")

    cand_send = nc.dram_tensor("cand_send", [QCH, 128, 16], f32)
    cand_recv = nc.dram_tensor("cand_recv", [NCORES, 128, 16], f32)

    tiles = _mtiles(mpad)
    with tile.TileContext(nc) as tc, ExitStack() as ctx:
        cpool = ctx.enter_context(tc.tile_pool(name="const", bufs=1))

        ident = cpool.tile([128, 128], f32)
        make_identity(nc, ident[:])

        # lam replicated across partitions
        lam_p = cpool.tile([128, 1], f32)
        nc.sync.dma_start(lam_p[:], bass.AP(lam_in, 0, [[0, 128], [1, 1]]))

        # qt per chunk: [128, QCH]
        qt_col = cpool.tile([128, QCH], f32)
        nc.sync.dma_start(qt_col[:], bass.AP(qtimes, 0, [[1, 128], [128, QCH]]))

        # ---- query normalization (all N) + transpose -----------------------
        q_all = cpool.tile([128, QCH, D], f32)
        nc.sync.dma_start(q_all[:], bass.AP(queries, 0,
                                            [[D, 128], [128 * D, QCH], [1, D]]))
        ssqq = cpool.tile([128, QCH], f32)
        with tc.tile_pool(name="qnorm", bufs=2) as qp:
            for c in range(QCH):
                scr = qp.tile([128, D], f32)
                nc.scalar.activation(out=scr[:], in_=q_all[:, c, :],
                                     func=AF.Square,
                                     accum_out=ssqq[:, c:c + 1])
            nc.scalar.activation(out=ssqq[:], in_=ssqq[:], func=AF.Sqrt)
            nc.vector.tensor_scalar(out=ssqq[:], in0=ssqq[:], scalar1=EPS,
                                    scalar2=None, op0=A.max)
            rq = cpool.tile([128, QCH], f32)
            nc.vector.reciprocal(out=rq[:], in_=ssqq[:])
            qn = cpool.tile([128, QCH, D], f32)
            for c in range(QCH):
                nc.vector.tensor_scalar(out=qn[:, c, :], in0=q_all[:, c, :],
                                        scalar1=rq[:, c:c + 1], scalar2=None,
                                        op0=A.mult)
        # qnT[h]: [128 (d half), N]
        qnT = [cpool.tile([128, N], f32, tag=f"qnT{h}", name=f"qnT{h}") for h in range(2)]
        with tc.tile_pool(name="qtr", bufs=2, space="PSUM") as qtp:
            for c in range(QCH):
                for h in range(2):
                    pst = qtp.tile([128, 128], f32, tag="pst")
                    nc.tensor.transpose(out=pst[:],
                                        in_=qn[:, c, h * 128:(h + 1) * 128],
                                        identity=ident[:])
                    nc.scalar.copy(out=qnT[h][:, c * 128:(c + 1) * 128],
                                   in_=pst[:])

        # per-chunk candidate keys
        candv = [cpool.tile([128, 8 * len(tiles)], f32, tag=f"candv{c}",
                            name=f"candv{c}") for c in range(QCH)]

        # ---- stage 1 main loop over m-tiles --------------------------------
        with tc.tile_pool(name="mt_sb", bufs=3) as mp, \
             tc.tile_pool(name="mt_sc", bufs=3) as sp, \
             tc.tile_pool(name="mt_ps", bufs=3, space="PSUM") as pp, \
             tc.tile_pool(name="mt_pt", bufs=2, space="PSUM") as tp:
            for ti, (off, S) in enumerate(tiles):
                nsub = S // 128
                rt = mp.tile([128, nsub, D], f32, tag="rt")
                nc.sync.dma_start(
                    rt[:], bass.AP(slab, off * D,
                                   [[D, 128], [128 * D, nsub], [1, D]]))
                ssq = mp.tile([128, nsub], f32, tag="ssq")
                for j in range(nsub):
                    scr = mp.tile([128, D], f32, tag="sqscr")
                    nc.scalar.activation(out=scr[:], in_=rt[:, j, :],
                                         func=AF.Square,
                                         accum_out=ssq[:, j:j + 1])
                nc.scalar.activation(out=ssq[:], in_=ssq[:], func=AF.Sqrt)
                nc.vector.tensor_scalar(out=ssq[:], in0=ssq[:], scalar1=EPS,
                                        scalar2=None, op0=A.max)
                rr = mp.tile([128, nsub], f32, tag="rr")
                nc.vector.reciprocal(out=rr[:], in_=ssq[:])
                for j in range(nsub):
                    nc.vector.tensor_scalar(out=rt[:, j, :], in0=rt[:, j, :],
                                            scalar1=rr[:, j:j + 1],
                                            scalar2=None, op0=A.mult)
                # transpose to [d, m] halves
                sT = [mp.tile([128, S], f32, tag=f"sT{h}", name=f"sT{h}") for h in range(2)]
                for j in range(nsub):
                    for h in range(2):
                        pst = tp.tile([128, 128], f32, tag="pst")
                        nc.tensor.transpose(
                            out=pst[:], in_=rt[:, j, h * 128:(h + 1) * 128],
                            identity=ident[:])
                        nc.scalar.copy(out=sT[h][:, j * 128:(j + 1) * 128],
                                       in_=pst[:])
                # replicated pool times / index codes
                ptrep = mp.tile([128, S], f32, tag="ptrep")
                nc.sync.dma_start(ptrep[:],
                                  bass.AP(slab_pt, off, [[0, 128], [1, S]]))
                idxrep = mp.tile([128, S], u32, tag="idxrep")
                nc.sync.dma_start(idxrep[:],
                                  bass.AP(idxcode, off, [[0, 128], [1, S]]))

                for c in range(QCH):
                    ps = pp.tile([128, S], f32, tag="ps")
                    for h in range(2):
                        nc.tensor.matmul(
                            out=ps[:],
                            lhsT=qnT[h][:, c * 128:(c + 1) * 128].bitcast(mm_dt),
                            rhs=sT[h][:].bitcast(mm_dt),
                            start=(h == 0), stop=(h == 1))
                    dtl = sp.tile([128, S], f32, tag="dtl")
                    nc.vector.tensor_scalar(
                        out=dtl[:], in0=ptrep[:], scalar1=qt_col[:, c:c + 1],
                        scalar2=lam_p[:, 0:1], op0=A.subtract, op1=A.mult)
                    nc.vector.tensor_scalar(
                        out=dtl[:].bitcast(u32), in0=dtl[:].bitcast(u32),
                        scalar1=0x7FFFFFFF, scalar2=None, op0=A.bitwise_and)
                    tw = sp.tile([128, S], f32, tag="tw")
                    nc.scalar.activation(out=tw[:], in_=dtl[:], func=AF.Exp,
                                         scale=-1.0)
                    sc = sp.tile([128, S], f32, tag="sc")
                    nc.vector.tensor_tensor(out=sc[:], in0=ps[:], in1=tw[:],
                                            op=A.mult)
                    kt = sp.tile([128, S], u32, tag="kt")
                    nc.vector.tensor_scalar(out=kt[:],
                                            in0=sc[:].bitcast(u32),
                                            scalar1=MASK_LOCAL, scalar2=None,
                                            op0=A.bitwise_and)
                    key = sp.tile([128, S], u32, tag="key")
                    nc.vector.tensor_tensor(out=key[:], in0=kt[:],
                                            in1=idxrep[:], op=A.bitwise_or)
                    nc.vector.max(out=candv[c][:, ti * 8:(ti + 1) * 8],
                                  in_=key[:].bitcast(f32))

        # ---- local top-16 merge + send ------------------------------------
        with tc.tile_pool(name="merge", bufs=2) as mg:
            for c in range(QCH):
                t8a = mg.tile([128, 8], f32, tag="t8a")
                nc.vector.max(out=t8a[:], in_=candv[c][:])
                kn = mg.tile([128, 8 * len(tiles)], f32, tag="kn")
                nc.vector.match_replace(out=kn[:], in_to_replace=t8a[:],
                                        in_values=candv[c][:], imm_value=NEG)
                t8b = mg.tile([128, 8], f32, tag="t8b")
                nc.vector.max(out=t8b[:], in_=kn[:])
                nc.sync.dma_start(cand_send[c, :, 0:8], t8a[:])
                nc.sync.dma_start(cand_send[c, :, 8:16], t8b[:])

        # ---- collective ----------------------------------------------------
        if use_cc:
            nc.gpsimd.collective_compute(
                "AllToAll", A.bypass, replica_groups=[list(range(NCORES))],
                ins=[cand_send[:]], outs=[cand_recv[:]])
        else:
            with tc.tile_pool(name="ccbounce", bufs=1) as cb:
                bt = cb.tile([128, QCH * 16], f32)
                nc.sync.dma_start(bt[:], bass.AP(cand_send, 0,
                                                 [[16, 128], [128 * 16, QCH],
                                                  [1, 16]]))
                nc.sync.dma_start(bass.AP(cand_recv, 0,
                                          [[16, 128], [128 * 16, QCH],
                                           [1, 16]]), bt[:])

        def _stage2_body():
            # ---- stage 2 -------------------------------------------------------
            s2 = ctx.enter_context(tc.tile_pool(name="s2", bufs=1))
            s2w = ctx.enter_context(tc.tile_pool(name="s2w", bufs=2))
            s2p = ctx.enter_context(tc.tile_pool(name="s2p", bufs=2, space="PSUM"))

            allk = s2.tile([128, NCORES * 16], u32)
            nc.sync.dma_start(
                allk[:], bass.AP(cand_recv, 0,
                                 [[16, 128], [128 * 16, NCORES],
                                  [1, 16]]).bitcast(u32))
            # repack: 14-bit local code -> 17-bit global code
            allk2 = s2.tile([128, NCORES * 16], u32)
            for s in range(NCORES):
                sl = slice(16 * s, 16 * s + 16)
                g = s2w.tile([128, 16], u32, tag="rp_g")
                nc.vector.tensor_scalar(out=g[:], in0=allk[:, sl],
                                        scalar1=0x3FFF, scalar2=None,
                                        op0=A.bitwise_and)
                nc.vector.tensor_scalar(out=g[:], in0=g[:], scalar1=0x3FFF,
                                        scalar2=None, op0=A.bitwise_xor)
                nc.vector.tensor_scalar(out=g[:], in0=g[:], scalar1=s * mloc,
                                        scalar2=None, op0=A.add)
                nc.vector.tensor_scalar(out=g[:], in0=g[:], scalar1=0x1FFFF,
                                        scalar2=None, op0=A.bitwise_xor)
                vb = s2w.tile([128, 16], u32, tag="rp_vb")
                nc.vector.tensor_scalar(out=vb[:], in0=allk[:, sl],
                                        scalar1=MASK_GLOBAL, scalar2=None,
                                        op0=A.bitwise_and)
                nc.vector.tensor_tensor(out=allk2[:, sl], in0=vb[:], in1=g[:],
                                        op=A.bitwise_or)

            # top-NCAND of union
            mk = s2.tile([128, ncand], f32)
            cur = allk2
            for r in range(ncand // 8):
                nc.vector.max(out=mk[:, r * 8:(r + 1) * 8],
                              in_=cur[:].bitcast(f32))
                if r < ncand // 8 - 1:
                    nxt = s2w.tile([128, NCORES * 16], u32, tag="mr_nxt")
                    nc.vector.match_replace(out=nxt[:].bitcast(f32),
                                            in_to_replace=mk[:, r * 8:(r + 1) * 8],
                                            in_values=cur[:].bitcast(f32),
                                            imm_value=NEG)
                    cur = nxt

            gidx = s2.tile([128, ncand], u32)
            nc.vector.tensor_scalar(out=gidx[:], in0=mk[:].bitcast(u32),
                                    scalar1=0x1FFFF, scalar2=0x1FFFF,
                                    op0=A.bitwise_and, op1=A.bitwise_xor)

            if s2depth < 2:
                nc.vector.tensor_copy(out=mk[:, 0:8], in_=gidx[:, 0:8])
                nc.sync.dma_start(out_d[:], mk[:, 0])
                return
            # gather candidate rows [128, ncand, AUGC]
            # NOTE: HW indirect DMA needs a standalone offset-0 [128,1] index
            # tile per instruction (sliced/multi-index offset APs misread).
            grows = s2.tile([128, ncand, AUGC], f32)
            for cnd in range(ncand):
                ixc = s2w.tile([128, 1], u32, tag="ixc")
                nc.vector.tensor_copy(out=ixc[:], in_=gidx[:, cnd:cnd + 1])
                nc.gpsimd.indirect_dma_start(
                    out=grows[:, cnd, :], out_offset=None, in_=pool_aug[:],
                    in_offset=bass.IndirectOffsetOnAxis(ap=ixc[:, 0:1],
                                                        axis=0))

            if s2depth < 21:
                nc.sync.dma_start(out_d[:], grows[:, 0, 0])
                return
            # qn2 = exact normalized query slice
            q2 = s2.tile([128, D], f32)
            nc.sync.dma_start(q2[:], qslice[:])
            qt2 = s2.tile([128, 1], f32)
            nc.sync.dma_start(qt2[:], bass.AP(qtslice, 0, [[1, 128], [1, 1]]))
            ssq2 = s2.tile([128, 1], f32)
            scr2 = s2.tile([128, D], f32)
            nc.scalar.activation(out=scr2[:], in_=q2[:], func=AF.Square,
                                 accum_out=ssq2[:, 0:1])
            nc.scalar.activation(out=ssq2[:], in_=ssq2[:], func=AF.Sqrt)
            nc.vector.tensor_scalar(out=ssq2[:], in0=ssq2[:], scalar1=EPS,
                                    scalar2=None, op0=A.max)
            rq2 = s2.tile([128, 1], f32)
            nc.vector.reciprocal(out=rq2[:], in_=ssq2[:])
            qn2 = s2.tile([128, D], f32)
            nc.vector.tensor_scalar(out=qn2[:], in0=q2[:], scalar1=rq2[:, 0:1],
                                    scalar2=None, op0=A.mult)

            # exact rescore
            dots = s2.tile([128, ncand], f32)
            ssqc = s2.tile([128, ncand], f32)
            for cnd in range(ncand):
                row = grows[:, cnd, 0:D]
                pr = s2w.tile([128, D], f32, tag="rs_pr")
                nc.vector.tensor_tensor(out=pr[:], in0=row, in1=qn2[:],
                                        op=A.mult)
                nc.vector.tensor_reduce(out=dots[:, cnd:cnd + 1], in_=pr[:],
                                        axis=mybir.AxisListType.X, op=A.add)
            if s2depth < 22:
                nc.sync.dma_start(out_d[:], dots[:, 0])
                return
            for cnd in range(ncand):
                row = grows[:, cnd, 0:D]
                pr2 = s2w.tile([128, D], f32, tag="rs_pr2")
                nc.vector.tensor_tensor(out=pr2[:], in0=row, in1=row,
                                        op=A.mult)
                nc.vector.tensor_reduce(out=ssqc[:, cnd:cnd + 1], in_=pr2[:],
                                        axis=mybir.AxisListType.X, op=A.add)
            nc.scalar.activation(out=ssqc[:], in_=ssqc[:], func=AF.Sqrt)
            nc.vector.tensor_scalar(out=ssqc[:], in0=ssqc[:], scalar1=EPS,
                                    scalar2=None, op0=A.max)
            rpc = s2.tile([128, ncand], f32)
            nc.vector.reciprocal(out=rpc[:], in_=ssqc[:])
            if s2depth < 23:
                nc.sync.dma_start(out_d[:], rpc[:, 0])
                return
            # time weights from gathered times (column D of aug rows)
            ptc = bass.AP(grows.tensor, grows[:].offset + D,
                          [grows[:].ap[0], [AUGC, ncand]])
            dc = s2.tile([128, ncand], f32)
            nc.vector.tensor_scalar(out=dc[:], in0=ptc, scalar1=qt2[:, 0:1],
                                    scalar2=lam_p[:, 0:1], op0=A.subtract,
                                    op1=A.mult)
            nc.vector.tensor_scalar(out=dc[:].bitcast(u32), in0=dc[:].bitcast(u32),
                                    scalar1=0x7FFFFFFF, scalar2=None,
                                    op0=A.bitwise_and)
            twc = s2.tile([128, ncand], f32)
            nc.scalar.activation(out=twc[:], in_=dc[:], func=AF.Exp,
                                 scale=-1.0)
            if s2depth < 24:
                nc.sync.dma_start(out_d[:], twc[:, 0])
                return
            sex = s2.tile([128, ncand], f32)
            nc.vector.tensor_tensor(out=sex[:], in0=dots[:], in1=rpc[:], op=A.mult)
            nc.vector.tensor_tensor(out=sex[:], in0=sex[:], in1=twc[:], op=A.mult)

            if s2depth < 30:
                nc.sync.dma_start(out_d[:], sex[:, 0])
                return
            # final top-7 (positions in candidate list)
            v8f = s2.tile([128, 8], f32)
            nc.vector.max(out=v8f[:], in_=sex[:])
            f8p = s2.tile([128, 8], u32)
            nc.vector.max_index(out=f8p[:], in_max=v8f[:], in_values=sex[:])

            # one-hot weights: w[p, c] = sum_k u_k/7 * [f8p[p,k] == c]
            iotac = s2.tile([128, ncand], f32)
            nc.sync.dma_start(iotac[:], bass.AP(iotaC, 0, [[0, 128], [1, ncand]]))
            f8pf = s2.tile([128, 8], f32)
            nc.vector.tensor_copy(out=f8pf[:], in_=f8p[:])
            wsel = s2.tile([128, ncand], f32)
            nc.vector.memset(wsel[:], 0.0)
            for k in range(K):
                wk = s2w.tile([128, ncand], f32, tag="wk")
                nc.vector.tensor_scalar(out=wk[:], in0=iotac[:],
                                        scalar1=f8pf[:, k:k + 1],
                                        scalar2=float(U_COEF[k]),
                                        op0=A.is_equal, op1=A.mult)
                nc.vector.tensor_tensor(out=wsel[:], in0=wsel[:], in1=wk[:],
                                        op=A.add)

            # dweight = sum_c w[p,c] * emb row c
            wg = s2.tile([128, ncand, D], f32)
            emb = bass.AP(grows.tensor, grows[:].offset,
                          [grows[:].ap[0], [AUGC, ncand], [1, D]])
            wb = bass.AP(wsel.tensor, wsel[:].offset,
                         [wsel[:].ap[0], [1, ncand], [0, D]])
            nc.vector.tensor_tensor(out=wg[:], in0=emb, in1=wb, op=A.mult)
            dw = s2.tile([128, D], f32)
            wgr = bass.AP(wg.tensor, wg[:].offset,
                          [wg[:].ap[0], [1, D], [D, ncand]])
            nc.vector.tensor_reduce(out=dw[:], in_=wgr, axis=mybir.AxisListType.X,
                                    op=A.add)

            if s2depth < 40:
                nc.sync.dma_start(out_d[:], dw[:, 0])
                return
            # h = dw @ W + b  (transpose, matmul, add bias, transpose back)
            W_sb = s2.tile([128, 2, D], f32)
            nc.sync.dma_start(W_sb[:], bass.AP(W_in, 0,
                                               [[D, 128], [128 * D, 2], [1, D]]))
            b_sb = s2.tile([128, 2], f32)
            nc.sync.dma_start(b_sb[:], bass.AP(b_in, 0, [[1, 128], [128, 2]]))
            dwT = [s2.tile([128, 128], f32, tag=f"dwT{h}", name=f"dwT{h}") for h in range(2)]
            for h in range(2):
                pst = s2p.tile([128, 128], f32, tag="s2pst")
                nc.tensor.transpose(out=pst[:], in_=dw[:, h * 128:(h + 1) * 128],
                                    identity=ident[:])
                nc.scalar.copy(out=dwT[h][:], in_=pst[:])
            fusedT = [s2.tile([128, 128], f32, tag=f"fusedT{h}", name=f"fusedT{h}")
                      for h in range(2)]
            for hh in range(2):
                psh = s2p.tile([128, 128], f32, tag="psh")
                for h in range(2):
                    nc.tensor.matmul(out=psh[:],
                                     lhsT=W_sb[:, h, hh * 128:(hh + 1) * 128],
                                     rhs=dwT[h][:], start=(h == 0), stop=(h == 1))
                nc.scalar.activation(out=fusedT[hh][:], in_=psh[:],
                                     func=AF.Identity, bias=b_sb[:, hh:hh + 1])
            fused = s2.tile([128, D], f32)
            for hh in range(2):
                pst = s2p.tile([128, 128], f32, tag="s2pst")
                nc.tensor.transpose(out=pst[:], in_=fusedT[hh][:],
                                    identity=ident[:])
                nc.scalar.copy(out=fused[:, hh * 128:(hh + 1) * 128], in_=pst[:])

            # out = ALPHA*(1-cos) + BETA*l2
            qmf = s2.tile([128, D], f32)
            nc.vector.tensor_tensor(out=qmf[:], in0=q2[:], in1=fused[:],
                                    op=A.subtract)
            t1 = s2.tile([128, 1], f32)
            t2 = s2.tile([128, 1], f32)
            t3 = s2.tile([128, 1], f32)
            for (tt, a0, a1) in ((t1, qn2, fused), (t2, fused, fused),
                                 (t3, qmf, qmf)):
                prx = s2w.tile([128, D], f32, tag="fin_pr")
                nc.vector.tensor_tensor(out=prx[:], in0=a0[:], in1=a1[:],
                                        op=A.mult)
                nc.vector.tensor_reduce(out=tt[:, 0:1], in_=prx[:],
                                        axis=mybir.AxisListType.X, op=A.add)
            nf = s2.tile([128, 1], f32)
            nc.scalar.activation(out=nf[:], in_=t2[:], func=AF.Sqrt)
            nc.vector.tensor_scalar(out=nf[:], in0=nf[:], scalar1=EPS,
                                    scalar2=None, op0=A.max)
            rnf = s2.tile([128, 1], f32)
            nc.vector.reciprocal(out=rnf[:], in_=nf[:])
            cosv = s2.tile([128, 1], f32)
            nc.vector.tensor_tensor(out=cosv[:], in0=t1[:], in1=rnf[:], op=A.mult)
            l2 = s2.tile([128, 1], f32)
            nc.scalar.activation(out=l2[:], in_=t3[:], func=AF.Sqrt)
            res = s2.tile([128, 1], f32)
            nc.vector.tensor_scalar(out=res[:], in0=cosv[:], scalar1=-ALPHA,
                                    scalar2=ALPHA, op0=A.mult, op1=A.add)
            acc = s2.tile([128, 1], f32)
            nc.vector.tensor_scalar(out=acc[:], in0=l2[:], scalar1=BETA,
                                    scalar2=None, op0=A.mult)
            nc.vector.tensor_tensor(out=res[:], in0=res[:], in1=acc[:], op=A.add)
            nc.sync.dma_start(out_d[:], res[:, 0])
        if stage2:
            _stage2_body()
        else:
            s2x = ctx.enter_context(tc.tile_pool(name="s2x", bufs=1))
            dbg = s2x.tile([128, 16], f32)
            nc.sync.dma_start(dbg[:], bass.AP(cand_recv, 0,
                                              [[16, 128], [1, 16]]).bitcast(f32))
            nc.sync.dma_start(out_d[:], dbg[:, 0])

    nc.compile()
    return nc


# ---- host wrapper ---------------------------------------------------------
_cached = {}


def _get_program(**kw):
    key = tuple(sorted(kw.items()))
    if key not in _cached:
        _cached[key] = build_program(**kw)
    return _cached[key]


def make_in_maps(query_emb, query_time, pool_emb, pool_time, lambda_decay,
                 W, b, mloc=MLOC, mpad=MPAD, ncand=NCAND):
    m = pool_emb.shape[0]
    aug = np.zeros((m, AUGC), np.float32)
    aug[:, :D] = pool_emb
    aug[:, D] = pool_time
    idxcode = (16383 - np.arange(mpad, dtype=np.uint32)).astype(np.uint32)
    iotac = np.arange(ncand, dtype=np.float32)
    lam = np.asarray(lambda_decay, np.float32).reshape(1)
    in_maps = []
    for c in range(NCORES):
        sl = np.zeros((mpad, D), np.float32)
        sp = np.zeros((mpad,), np.float32)
        sl[:mloc] = pool_emb[c * mloc:(c + 1) * mloc]
        sp[:mloc] = pool_time[c * mloc:(c + 1) * mloc]
        in_maps.append({
            "slab": sl, "slab_pt": sp, "pool_aug": aug,
            "queries": np.ascontiguousarray(query_emb, dtype=np.float32),
            "qtimes": np.ascontiguousarray(query_time, dtype=np.float32),
            "qslice": np.ascontiguousarray(query_emb[c * 128:(c + 1) * 128]),
            "qtslice": np.ascontiguousarray(query_time[c * 128:(c + 1) * 128]),
            "lam": lam, "W": np.ascontiguousarray(W, dtype=np.float32),
            "b": np.ascontiguousarray(b, dtype=np.float32),
            "idxcode": idxcode, "iotaC": iotac,
        })
    return in_maps


def kernel(query_emb, query_time, pool_emb, pool_time, lambda_decay, W, b):
    from concourse.bass_utils import run_bass_kernel_spmd
    nc = _get_program()
    in_maps = make_in_maps(np.asarray(query_emb), np.asarray(query_time),
                           np.asarray(pool_emb), np.asarray(pool_time),
                           np.asarray(lambda_decay), np.asarray(W),
                           np.asarray(b))
    res = run_bass_kernel_spmd(nc, in_maps, core_ids=list(range(NCORES)))
    return np.concatenate([res.results[c]["out"] for c in range(NCORES)])


if __name__ == "__main__":
    print("building program...")
    nc = build_program()
    print("ok:", len(nc.m.functions[0].instructions) if hasattr(nc.m.functions[0], "instructions") else "built")

